# revision 44
# baseline (speedup 1.0000x reference)
"""Trainium2 Bass kernel for nn_CustomLinearFullFP8.

y = (fp8e4m3fn(x / sx) @ fp8e4m3fn(W / sW).T) * sx * sW,
  sx = amax(|x|)/448, sW = amax(|W|)/448, accumulation fp32.

Strategy (8 NeuronCores, data-parallel over M):
- Host transposes x so K lands on the SBUF partition axis; each core gets
  xT shard [512, 16384] plus the replicated WT [512, 512].
- Pass 1: stream xT once; DVE computes per-chunk amax from the fp32 data
  (exact), Act converts each chunk to fp16 residing in SBUF (16 MiB - all
  32 chunks stay resident, no re-read).
- Core amax: gpsimd partition_all_reduce -> [1,1] -> AllGather(8) -> local
  max (the cost model charges AllReduce 1.875x the AllGather constant).
- W path (off the collective's critical path): W streams last so its DMA
  fills the collective bubble; local amax + fp8 quantization as baseline.
- Pass 2: quantize fp16 residents to TRN fp8e4 with scale 224/amax (TRN
  e4m3 saturates at 240 -> half-scale quantization, exact on the e4m3fn
  grid; the factor 4 folds into the output scale), DoubleRow fp8 matmuls,
  evacuate PSUM with scale amax_x*amax_W/50176 into fp16 (split across
  Act/DVE/Pool), DMA y out as fp16 (host upcasts to fp32).
"""

import os

import numpy as np

import concourse.bass as bass
import concourse.bacc as bacc
import concourse.mybir as mybir
import concourse.tile as tile
from concourse import bass_isa
from concourse.bass_utils import run_bass_kernel_spmd

F32 = mybir.dt.float32
F16 = mybir.dt.float16
FP8 = mybir.dt.float8e4
AF = mybir.ActivationFunctionType
AX = mybir.AxisListType

N_CORES = 8
M_FULL, K, N = 131072, 512, 512
M_SH = M_FULL // N_CORES          # 16384 rows per core
KC = K // 128                     # 4 k-subtiles
MT = 512                          # m-chunk size (512 rows -> 4 psum banks)
N_CHUNKS = M_SH // MT             # 32
XS_BUFS = int(os.environ.get("KXS", "3"))
LOOKAHEAD = int(os.environ.get("KLA", "3"))   # quant emission lookahead (chunks)
XQ_BUFS = int(os.environ.get("KXQ", str(LOOKAHEAD + 2)))
YS_BUFS = int(os.environ.get("KYS", "5"))
PS_BUFS = int(os.environ.get("KPS", "2"))     # [128,4,512] f32 = 4 banks each
# evac engine split by column: act takes [0:ACT_COLS), pool/dve the rest
ACT_COLS = int(os.environ.get("KAC", "1536"))
POOL_EVAC = os.environ.get("KPOOL", "1") == "1"
N_WARMUP = int(os.environ.get("KNW", "46"))   # PE warmup matmuls in the bubble
Y_DTYPE = os.environ.get("KYD", "f16")        # f16|f32
X_RES_DTYPE = os.environ.get("KXD", "f16")    # f16|f32 (f32 only for debug)

_cached_nc = None


def build_bass():
    ydt = F16 if Y_DTYPE == "f16" else F32
    xdt = F16 if X_RES_DTYPE == "f16" else F32
    nc = bacc.Bacc(None, target_bir_lowering=False, debug=False, num_devices=N_CORES)
    xt = nc.dram_tensor("xt", [N_CHUNKS, 128, KC * MT], F32, kind="ExternalInput")
    wt = nc.dram_tensor("wt", [K, N], F32, kind="ExternalInput")
    y = nc.dram_tensor("y", [N_CHUNKS, 128, 4 * N], ydt, kind="ExternalOutput")

    wt3 = wt.rearrange("(c p) n -> p c n", p=128)   # [128, 4, N]

    with tile.TileContext(nc) as tc:
        with (
            tc.tile_pool(name="xres", bufs=1) as xres_pool,
            tc.tile_pool(name="xstream", bufs=XS_BUFS) as xstream_pool,
            tc.tile_pool(name="xq", bufs=XQ_BUFS) as xq_pool,
            tc.tile_pool(name="ystagea", bufs=YS_BUFS) as ya_pool,
            tc.tile_pool(name="ystageb", bufs=YS_BUFS) as yb_pool,
            tc.tile_pool(name="cst", bufs=1) as cst,
            tc.tile_pool(name="psuma", bufs=PS_BUFS, space="PSUM") as psa_pool,
            tc.tile_pool(name="psumb", bufs=PS_BUFS, space="PSUM") as psb_pool,
            tc.tile_pool(name="dram", bufs=2, space="DRAM") as dram,
        ):
            # ---- resident fp16 x tiles (live whole kernel)
            xres = [
                xres_pool.tile([128, KC, MT], xdt, tag=f"xres{i}", name=f"xres{i}")
                for i in range(N_CHUNKS)
            ]

            # ---- pass 1: stream x once; amax (DVE, fp32-exact) + fp16 convert
            # The last PIECED chunks stream in 128-row pieces: a full-chunk
            # reduce (2194ns) only starts after its whole DMA (2913ns), so at
            # the stream's end DVE would trail by ~2.2us; with pieces the
            # reduces pipeline against the DMA and only ~0.6us trails, moving
            # the collective's start earlier.
            PIECED = 3
            amax_parts = cst.tile([128, N_CHUNKS + 3 * PIECED], F32)
            xt4 = xt.rearrange("i p (c m) -> i p c m", c=KC)
            col = 0
            for i in range(N_CHUNKS):
                xtile = xstream_pool.tile([128, KC, MT], F32, tag="xs",
                                          name=f"xs{i}")
                if i < N_CHUNKS - PIECED:
                    nc.sync.dma_start(
                        xtile[:].rearrange("p c m -> p (c m)"), xt[i])
                    nc.vector.reduce_max(amax_parts[:, col:col + 1], xtile[:],
                                         axis=AX.XY, apply_absolute_value=True)
                    col += 1
                else:
                    for q in range(4):
                        sl = slice(q * (MT // 4), (q + 1) * (MT // 4))
                        nc.sync.dma_start(xtile[:, :, sl], xt4[i, :, :, sl])
                        nc.vector.reduce_max(amax_parts[:, col:col + 1],
                                             xtile[:, :, sl], axis=AX.XY,
                                             apply_absolute_value=True)
                        col += 1
                nc.scalar.activation(xres[i][:], xtile[:], AF.Copy)
            assert col == N_CHUNKS + 3 * PIECED

            # core-local amax -> all partitions, then [1,1] to DRAM
            pk2 = cst.tile([128, 1], F32)
            nc.vector.reduce_max(pk2[:, 0:1], amax_parts[:], axis=AX.X)
            axall = cst.tile([128, 1], F32)
            nc.gpsimd.partition_all_reduce(axall[:], pk2[:], 128,
                                           bass_isa.ReduceOp.max)
            cc_in = dram.tile([1, 1], F32)
            cc_out = dram.tile([1, N_CORES], F32)
            nc.sync.dma_start(cc_in[:], axall[0:1, 0:1])
            nc.gpsimd.collective_compute(
                "AllGather", mybir.AluOpType.bypass,
                replica_groups=[list(range(N_CORES))],
                ins=[cc_in.opt()], outs=[cc_out.opt()],
            )
            g8 = cst.tile([1, N_CORES], F32)
            nc.sync.dma_start(g8[:], cc_out[:])

            # ---- W load + quant: issued after the x stream so its DMA and
            # compute land in the collective bubble (off the critical path)
            wt_sb = cst.tile([128, 4, N], F32, name="wt_sb")
            nc.sync.dma_start(wt_sb[:], wt3[:])
            awmax = cst.tile([128, 1], F32)
            nc.vector.reduce_max(awmax[:], wt_sb[:], axis=AX.XY,
                                 apply_absolute_value=True)
            awall = cst.tile([128, 1], F32)
            nc.gpsimd.partition_all_reduce(awall[:], awmax[:], 128,
                                           bass_isa.ReduceOp.max)
            rw = cst.tile([128, 1], F32)
            nc.vector.reciprocal(rw[0:1, 0:1], awall[0:1, 0:1])
            cwp = cst.tile([128, 1], F32)
            nc.vector.tensor_scalar_mul(cwp[0:1, 0:1], rw[0:1, 0:1], 224.0)
            cwb_t = cst.tile([128, 1], F32)
            nc.gpsimd.partition_broadcast(cwb_t[:], cwp[0:1, 0:1])
            wq = cst.tile([128, KC, N], FP8)
            nc.scalar.activation(wq[:], wt_sb[:], AF.Copy, scale=cwb_t[:, 0:1])

            # ---- global amax + packed scales: pk = [224/ax, ax*aw/50176]
            gx = cst.tile([1, 1], F32)
            nc.vector.reduce_max(gx[0:1, 0:1], g8[0:1, :], axis=AX.X)
            rec = cst.tile([1, 1], F32)
            nc.vector.reciprocal(rec[:], gx[:])
            pk = cst.tile([1, 2], F32)
            nc.vector.tensor_scalar_mul(pk[0:1, 0:1], rec[:], 224.0)
            nc.vector.tensor_scalar(pk[0:1, 1:2], gx[:], awall[0:1, 0:1],
                                    1.0 / 50176.0,
                                    mybir.AluOpType.mult,
                                    mybir.AluOpType.mult)
            bc4 = cst.tile([128, 2], F32)
            nc.gpsimd.partition_broadcast(bc4[:, 0:2], pk[0:1, 0:2])
            cxb = bc4[:, 0:1]
            cxb_p = bc4
            osb = bc4[:, 1:2]
            osb_d = bc4[:, 1:2]

            # ---- PE warmup: dummy fp16 matmuls gated on the last x chunk.
            # They run back-to-back through the collective bubble so the
            # p-state ramp completes before the first real matmul.
            if N_WARMUP:
                wps = psa_pool.tile([128, 3, N], F32, tag="psa", name="warm")
                for _ in range(N_WARMUP):
                    nc.tensor.matmul(wps[:, 0, :], xres[N_CHUNKS - 1][:, 0, 0:128],
                                     xres[N_CHUNKS - 1][:, 0, 0:N],
                                     start=True, stop=True)

            # ---- pass 2: quantize residents, matmul, evac with scale, DMA out
            # Quants are emitted LOOKAHEAD chunks ahead of their matmuls: the
            # tile scheduler lowers buffer-reuse waits as cumulative per-engine
            # counters in program order, so a quant emitted after chunk i's
            # matmuls would stall on them even with free xq buffers.
            xqs = {}
            MQ = MT // 2   # m-rows quantized by DVE; pool quantizes the rest
            # chunks at the pipeline fill/drain boundary quantize entirely on
            # DVE: the act evac spans both quant halves, so the slower pool
            # quant would otherwise sit on the first/last chunk's latency path
            DVE_ONLY = {0, 1, N_CHUNKS - 2, N_CHUNKS - 1}

            def emit_quant(j):
                # quant split across DVE and Pool (both SBUF->SBUF; gpsimd
                # cannot touch PSUM so its steady-state job is quantization).
                # Separate tiles per writer engine.
                if j in DVE_ONLY:
                    if j == 0:
                        # two half tiles: banks 0-1's matmuls start after the
                        # first 594ns quant instead of the full 1127ns one
                        xa = xq_pool.tile([128, KC, MQ], FP8, tag="xq0a",
                                          name="xq0a", bufs=1)
                        nc.vector.tensor_scalar_mul(xa[:], xres[j][:, :, 0:MQ],
                                                    cxb)
                        xb = xq_pool.tile([128, KC, MT - MQ], FP8, tag="xq0b",
                                          name="xq0b", bufs=1)
                        nc.vector.tensor_scalar_mul(xb[:],
                                                    xres[j][:, :, MQ:MT], cxb)
                        xqs[j] = (xa, xb)
                        return
                    xd = xq_pool.tile([128, KC, MT], FP8, tag="xqf",
                                      name=f"xqf{j}")
                    nc.vector.tensor_scalar_mul(xd[:], xres[j][:], cxb)
                    xqs[j] = (xd, None)
                    return
                xd = xq_pool.tile([128, KC, MQ], FP8, tag="xqd", name=f"xqd{j}")
                nc.vector.tensor_scalar_mul(xd[:], xres[j][:, :, 0:MQ], cxb)
                xp = xq_pool.tile([128, KC, MT - MQ], FP8, tag="xqp",
                                  name=f"xqp{j}")
                nc.gpsimd.tensor_scalar_mul(xp[:], xres[j][:, :, MQ:MT], cxb)
                xqs[j] = (xd, xp)

            for j in range(min(LOOKAHEAD, N_CHUNKS)):
                emit_quant(j)
            for i in range(N_CHUNKS):
                if i + LOOKAHEAD < N_CHUNKS:
                    emit_quant(i + LOOKAHEAD)
                xd, xp = xqs.pop(i)
                ab = ACT_COLS // N  # act's evac banks; DVE takes the rest
                # separate PSUM tiles per evac engine: PSUM accesses are
                # tracked at tile granularity and serialize, so a shared tile
                # would chain the second evac behind the first every chunk
                ps_a = psa_pool.tile([128, ab, N], F32, tag="psa")
                ps_b = psb_pool.tile([128, 4 - ab, N], F32, tag="psb")
                for jj in range(4):
                    out = ps_a[:, jj, :] if jj < ab else ps_b[:, jj - ab, :]
                    if xp is None:
                        src, m0 = xd, jj * 128
                    else:
                        src, m0 = (xd, jj * 128) if jj * 128 < MQ else \
                                  (xp, jj * 128 - MQ)
                    for kk in range(KC // 2):
                        nc.tensor.matmul(
                            out,
                            src[:, 2 * kk:2 * kk + 2, m0:m0 + 128],
                            wq[:, 2 * kk:2 * kk + 2, :],
                            start=(kk == 0), stop=(kk == KC // 2 - 1),
                            perf_mode=mybir.MatmulPerfMode.DoubleRow,
                        )
                yrow = y[i]  # [128, 4*N] in ydt
                # DVE's smaller evac+DMA goes first so the chunk (and the
                # kernel, on the last iteration) ends on the act-side DMA
                # rather than queueing the small transfer behind it
                yb = yb_pool.tile([128, 4 - ab, N], ydt, tag="ystb")
                nc.vector.tensor_scalar_mul(yb[:], ps_b[:], osb_d)
                nc.sync.dma_start(yrow[:, ab * N:],
                                  yb[:].rearrange("p b n -> p (b n)"))
                if i == N_CHUNKS - 1:
                    # last chunk: evacuate act's banks in two pieces so the
                    # kernel ends on a 364ns DMA whose 728ns predecessor
                    # overlapped the second evac, instead of one 1092ns DMA
                    # strictly after the full 1465ns evac
                    ya1 = ya_pool.tile([128, 2, N], ydt, tag="yspl_a", bufs=1)
                    nc.scalar.activation(ya1[:], ps_a[:, 0:2, :], AF.Copy,
                                         scale=osb)
                    nc.sync.dma_start(yrow[:, 0:2 * N],
                                      ya1[:].rearrange("p b n -> p (b n)"))
                    ya2 = ya_pool.tile([128, 1, N], ydt, tag="yspl_b", bufs=1)
                    nc.scalar.activation(ya2[:], ps_a[:, 2:3, :], AF.Copy,
                                         scale=osb)
                    nc.sync.dma_start(yrow[:, 2 * N:3 * N],
                                      ya2[:].rearrange("p b n -> p (b n)"))
                else:
                    ya = ya_pool.tile([128, ab, N], ydt, tag="ysta")
                    nc.scalar.activation(ya[:], ps_a[:], AF.Copy, scale=osb)
                    nc.sync.dma_start(yrow[:, 0:ab * N],
                                      ya[:].rearrange("p b n -> p (b n)"))
    nc.compile()
    return nc


def _get_nc():
    global _cached_nc
    if _cached_nc is None:
        _cached_nc = build_bass()
    return _cached_nc


def _make_in_maps(x: np.ndarray, W: np.ndarray):
    wt = np.ascontiguousarray(W.T)                # [K, N]
    # xt_blk[i, p, c*MT+m] = x[core*M_SH + i*MT + m, c*128 + p]
    xs = x.reshape(N_CORES, N_CHUNKS, MT, KC, 128)
    in_maps = []
    for c in range(N_CORES):
        blk = np.ascontiguousarray(
            xs[c].transpose(0, 3, 2, 1).reshape(N_CHUNKS, 128, KC * MT))
        in_maps.append({"xt": blk, "wt": wt})
    return in_maps


def kernel(x: np.ndarray, W: np.ndarray) -> np.ndarray:
    x = np.ascontiguousarray(x, dtype=np.float32)
    W = np.ascontiguousarray(W, dtype=np.float32)
    assert x.shape == (M_FULL, K) and W.shape == (N, K)

    in_maps = _make_in_maps(x, W)
    nc = _get_nc()
    res = run_bass_kernel_spmd(nc, in_maps, core_ids=list(range(N_CORES)))
    # y_blk[g, p, b*N+n] = y[g*512 + b*128 + p, n]
    outs = []
    for r in res.results:
        yb = r["y"].astype(np.float32).reshape(N_CHUNKS, 128, 4, N)
        outs.append(yb.transpose(0, 2, 1, 3).reshape(M_SH, N))
    return np.ascontiguousarray(np.concatenate(outs, axis=0),
                                dtype=np.float32)


# revision 46
# speedup vs baseline: 1.0001x; 1.0001x over previous
"""Trainium2 Bass kernel for nn_CustomLinearFullFP8.

y = (fp8e4m3fn(x / sx) @ fp8e4m3fn(W / sW).T) * sx * sW,
  sx = amax(|x|)/448, sW = amax(|W|)/448, accumulation fp32.

Strategy (8 NeuronCores, data-parallel over M):
- Host transposes x so K lands on the SBUF partition axis; each core gets
  xT shard [512, 16384] plus the replicated WT [512, 512].
- Pass 1: stream xT once; DVE computes per-chunk amax from the fp32 data
  (exact), Act converts each chunk to fp16 residing in SBUF (16 MiB - all
  32 chunks stay resident, no re-read).
- Core amax: gpsimd partition_all_reduce -> [1,1] -> AllGather(8) -> local
  max (the cost model charges AllReduce 1.875x the AllGather constant).
- W path (off the collective's critical path): W streams last so its DMA
  fills the collective bubble; local amax + fp8 quantization as baseline.
- Pass 2: quantize fp16 residents to TRN fp8e4 with scale 224/amax (TRN
  e4m3 saturates at 240 -> half-scale quantization, exact on the e4m3fn
  grid; the factor 4 folds into the output scale), DoubleRow fp8 matmuls,
  evacuate PSUM with scale amax_x*amax_W/50176 into fp16 (split across
  Act/DVE/Pool), DMA y out as fp16 (host upcasts to fp32).
"""

import os

import numpy as np

import concourse.bass as bass
import concourse.bacc as bacc
import concourse.mybir as mybir
import concourse.tile as tile
from concourse import bass_isa
from concourse.bass_utils import run_bass_kernel_spmd

F32 = mybir.dt.float32
F16 = mybir.dt.float16
FP8 = mybir.dt.float8e4
AF = mybir.ActivationFunctionType
AX = mybir.AxisListType

N_CORES = 8
M_FULL, K, N = 131072, 512, 512
M_SH = M_FULL // N_CORES          # 16384 rows per core
KC = K // 128                     # 4 k-subtiles
MT = 512                          # m-chunk size (512 rows -> 4 psum banks)
N_CHUNKS = M_SH // MT             # 32
XS_BUFS = int(os.environ.get("KXS", "3"))
LOOKAHEAD = int(os.environ.get("KLA", "3"))   # quant emission lookahead (chunks)
XQ_BUFS = int(os.environ.get("KXQ", str(LOOKAHEAD + 2)))
YS_BUFS = int(os.environ.get("KYS", "5"))
PS_BUFS = int(os.environ.get("KPS", "2"))     # [128,4,512] f32 = 4 banks each
# evac engine split by column: act takes [0:ACT_COLS), pool/dve the rest
ACT_COLS = int(os.environ.get("KAC", "1536"))
POOL_EVAC = os.environ.get("KPOOL", "1") == "1"
N_WARMUP = int(os.environ.get("KNW", "46"))   # PE warmup matmuls in the bubble
Y_DTYPE = os.environ.get("KYD", "f16")        # f16|f32
X_RES_DTYPE = os.environ.get("KXD", "f16")    # f16|f32 (f32 only for debug)

_cached_nc = None


def build_bass():
    ydt = F16 if Y_DTYPE == "f16" else F32
    xdt = F16 if X_RES_DTYPE == "f16" else F32
    nc = bacc.Bacc(None, target_bir_lowering=False, debug=False, num_devices=N_CORES)
    xt = nc.dram_tensor("xt", [N_CHUNKS, 128, KC * MT], F32, kind="ExternalInput")
    wt = nc.dram_tensor("wt", [K, N], F32, kind="ExternalInput")
    y = nc.dram_tensor("y", [N_CHUNKS, 128, 4 * N], ydt, kind="ExternalOutput")

    wt3 = wt.rearrange("(c p) n -> p c n", p=128)   # [128, 4, N]

    with tile.TileContext(nc) as tc:
        with (
            tc.tile_pool(name="xres", bufs=1) as xres_pool,
            tc.tile_pool(name="xstream", bufs=XS_BUFS) as xstream_pool,
            tc.tile_pool(name="xq", bufs=XQ_BUFS) as xq_pool,
            tc.tile_pool(name="ystagea", bufs=YS_BUFS) as ya_pool,
            tc.tile_pool(name="ystageb", bufs=YS_BUFS) as yb_pool,
            tc.tile_pool(name="cst", bufs=1) as cst,
            tc.tile_pool(name="psuma", bufs=PS_BUFS, space="PSUM") as psa_pool,
            tc.tile_pool(name="psumb", bufs=PS_BUFS, space="PSUM") as psb_pool,
            tc.tile_pool(name="dram", bufs=2, space="DRAM") as dram,
        ):
            # ---- resident fp16 x tiles (live whole kernel)
            xres = [
                xres_pool.tile([128, KC, MT], xdt, tag=f"xres{i}", name=f"xres{i}")
                for i in range(N_CHUNKS)
            ]

            # ---- pass 1: stream x once; amax (DVE, fp32-exact) + fp16 convert
            # The last PIECED chunks stream in 128-row pieces: a full-chunk
            # reduce (2194ns) only starts after its whole DMA (2913ns), so at
            # the stream's end DVE would trail by ~2.2us; with pieces the
            # reduces pipeline against the DMA and only ~0.6us trails, moving
            # the collective's start earlier.
            PIECED = 3
            amax_parts = cst.tile([128, N_CHUNKS + 3 * PIECED], F32)
            xt4 = xt.rearrange("i p (c m) -> i p c m", c=KC)
            col = 0
            for i in range(N_CHUNKS):
                xtile = xstream_pool.tile([128, KC, MT], F32, tag="xs",
                                          name=f"xs{i}")
                if i < N_CHUNKS - PIECED:
                    nc.sync.dma_start(
                        xtile[:].rearrange("p c m -> p (c m)"), xt[i])
                    nc.vector.reduce_max(amax_parts[:, col:col + 1], xtile[:],
                                         axis=AX.XY, apply_absolute_value=True)
                    col += 1
                else:
                    for q in range(4):
                        sl = slice(q * (MT // 4), (q + 1) * (MT // 4))
                        nc.sync.dma_start(xtile[:, :, sl], xt4[i, :, :, sl])
                        nc.vector.reduce_max(amax_parts[:, col:col + 1],
                                             xtile[:, :, sl], axis=AX.XY,
                                             apply_absolute_value=True)
                        col += 1
                nc.scalar.activation(xres[i][:], xtile[:], AF.Copy)
            assert col == N_CHUNKS + 3 * PIECED

            # core-local amax -> all partitions, then [1,1] to DRAM
            pk2 = cst.tile([128, 1], F32)
            nc.vector.reduce_max(pk2[:, 0:1], amax_parts[:], axis=AX.X)
            axall = cst.tile([128, 1], F32)
            nc.gpsimd.partition_all_reduce(axall[:], pk2[:], 128,
                                           bass_isa.ReduceOp.max)
            cc_in = dram.tile([1, 1], F32)
            cc_out = dram.tile([1, N_CORES], F32)
            nc.sync.dma_start(cc_in[:], axall[0:1, 0:1])
            nc.gpsimd.collective_compute(
                "AllGather", mybir.AluOpType.bypass,
                replica_groups=[list(range(N_CORES))],
                ins=[cc_in.opt()], outs=[cc_out.opt()],
            )
            # broadcast-read the gathered amaxes to every partition so the
            # whole scale chain runs per-partition on DVE - no cross-partition
            # gpsimd broadcast hop on the critical path
            g8 = cst.tile([128, N_CORES], F32, name="g8")
            nc.sync.dma_start(g8[:], cc_out[0:1, :].partition_broadcast(128))

            # ---- W load + quant: issued after the x stream so its DMA and
            # compute land in the collective bubble (off the critical path)
            wt_sb = cst.tile([128, 4, N], F32, name="wt_sb")
            nc.sync.dma_start(wt_sb[:], wt3[:])
            awmax = cst.tile([128, 1], F32)
            nc.vector.reduce_max(awmax[:], wt_sb[:], axis=AX.XY,
                                 apply_absolute_value=True)
            awall = cst.tile([128, 1], F32)
            nc.gpsimd.partition_all_reduce(awall[:], awmax[:], 128,
                                           bass_isa.ReduceOp.max)
            rw = cst.tile([128, 1], F32)
            nc.vector.reciprocal(rw[0:1, 0:1], awall[0:1, 0:1])
            cwp = cst.tile([128, 1], F32)
            nc.vector.tensor_scalar_mul(cwp[0:1, 0:1], rw[0:1, 0:1], 224.0)
            cwb_t = cst.tile([128, 1], F32)
            nc.gpsimd.partition_broadcast(cwb_t[:], cwp[0:1, 0:1])
            wq = cst.tile([128, KC, N], FP8)
            nc.scalar.activation(wq[:], wt_sb[:], AF.Copy, scale=cwb_t[:, 0:1])

            # ---- global amax + packed scales: bc4 = [224/ax, ax*aw/50176],
            # computed on all partitions directly (awall is per-partition)
            gxa = cst.tile([128, 1], F32, name="gxa")
            nc.vector.reduce_max(gxa[:], g8[:], axis=AX.X)
            reca = cst.tile([128, 1], F32, name="reca")
            nc.vector.reciprocal(reca[:], gxa[:])
            bc4 = cst.tile([128, 2], F32)
            nc.vector.tensor_scalar_mul(bc4[:, 0:1], reca[:], 224.0)
            nc.vector.tensor_scalar(bc4[:, 1:2], gxa[:], awall[:, 0:1],
                                    1.0 / 50176.0,
                                    mybir.AluOpType.mult,
                                    mybir.AluOpType.mult)
            cxb = bc4[:, 0:1]
            cxb_p = bc4
            osb = bc4[:, 1:2]
            osb_d = bc4[:, 1:2]

            # ---- PE warmup: dummy fp16 matmuls gated on the last x chunk.
            # They run back-to-back through the collective bubble so the
            # p-state ramp completes before the first real matmul.
            if N_WARMUP:
                wps = psa_pool.tile([128, 3, N], F32, tag="psa", name="warm")
                for _ in range(N_WARMUP):
                    nc.tensor.matmul(wps[:, 0, :], xres[N_CHUNKS - 1][:, 0, 0:128],
                                     xres[N_CHUNKS - 1][:, 0, 0:N],
                                     start=True, stop=True)

            # ---- pass 2: quantize residents, matmul, evac with scale, DMA out
            # Quants are emitted LOOKAHEAD chunks ahead of their matmuls: the
            # tile scheduler lowers buffer-reuse waits as cumulative per-engine
            # counters in program order, so a quant emitted after chunk i's
            # matmuls would stall on them even with free xq buffers.
            xqs = {}
            MQ = MT // 2   # m-rows quantized by DVE; pool quantizes the rest
            # chunks at the pipeline fill/drain boundary quantize entirely on
            # DVE: the act evac spans both quant halves, so the slower pool
            # quant would otherwise sit on the first/last chunk's latency path
            DVE_ONLY = {0, 1, N_CHUNKS - 2, N_CHUNKS - 1}

            def emit_quant(j):
                # quant split across DVE and Pool (both SBUF->SBUF; gpsimd
                # cannot touch PSUM so its steady-state job is quantization).
                # Separate tiles per writer engine.
                if j in DVE_ONLY:
                    if j == 0:
                        # two half tiles: banks 0-1's matmuls start after the
                        # first 594ns quant instead of the full 1127ns one
                        xa = xq_pool.tile([128, KC, MQ], FP8, tag="xq0a",
                                          name="xq0a", bufs=1)
                        nc.vector.tensor_scalar_mul(xa[:], xres[j][:, :, 0:MQ],
                                                    cxb)
                        xb = xq_pool.tile([128, KC, MT - MQ], FP8, tag="xq0b",
                                          name="xq0b", bufs=1)
                        nc.vector.tensor_scalar_mul(xb[:],
                                                    xres[j][:, :, MQ:MT], cxb)
                        xqs[j] = (xa, xb)
                        return
                    xd = xq_pool.tile([128, KC, MT], FP8, tag="xqf",
                                      name=f"xqf{j}")
                    nc.vector.tensor_scalar_mul(xd[:], xres[j][:], cxb)
                    xqs[j] = (xd, None)
                    return
                xd = xq_pool.tile([128, KC, MQ], FP8, tag="xqd", name=f"xqd{j}")
                nc.vector.tensor_scalar_mul(xd[:], xres[j][:, :, 0:MQ], cxb)
                xp = xq_pool.tile([128, KC, MT - MQ], FP8, tag="xqp",
                                  name=f"xqp{j}")
                nc.gpsimd.tensor_scalar_mul(xp[:], xres[j][:, :, MQ:MT], cxb)
                xqs[j] = (xd, xp)

            for j in range(min(LOOKAHEAD, N_CHUNKS)):
                emit_quant(j)
            for i in range(N_CHUNKS):
                if i + LOOKAHEAD < N_CHUNKS:
                    emit_quant(i + LOOKAHEAD)
                xd, xp = xqs.pop(i)
                ab = ACT_COLS // N  # act's evac banks; DVE takes the rest
                # separate PSUM tiles per evac engine: PSUM accesses are
                # tracked at tile granularity and serialize, so a shared tile
                # would chain the second evac behind the first every chunk
                ps_a = psa_pool.tile([128, ab, N], F32, tag="psa")
                ps_b = psb_pool.tile([128, 4 - ab, N], F32, tag="psb")
                for jj in range(4):
                    out = ps_a[:, jj, :] if jj < ab else ps_b[:, jj - ab, :]
                    if xp is None:
                        src, m0 = xd, jj * 128
                    else:
                        src, m0 = (xd, jj * 128) if jj * 128 < MQ else \
                                  (xp, jj * 128 - MQ)
                    for kk in range(KC // 2):
                        nc.tensor.matmul(
                            out,
                            src[:, 2 * kk:2 * kk + 2, m0:m0 + 128],
                            wq[:, 2 * kk:2 * kk + 2, :],
                            start=(kk == 0), stop=(kk == KC // 2 - 1),
                            perf_mode=mybir.MatmulPerfMode.DoubleRow,
                        )
                yrow = y[i]  # [128, 4*N] in ydt
                # DVE's smaller evac+DMA goes first so the chunk (and the
                # kernel, on the last iteration) ends on the act-side DMA
                # rather than queueing the small transfer behind it
                yb = yb_pool.tile([128, 4 - ab, N], ydt, tag="ystb")
                nc.vector.tensor_scalar_mul(yb[:], ps_b[:], osb_d)
                nc.sync.dma_start(yrow[:, ab * N:],
                                  yb[:].rearrange("p b n -> p (b n)"))
                if i == N_CHUNKS - 1:
                    # last chunk: evacuate act's banks in two pieces so the
                    # kernel ends on a 364ns DMA whose 728ns predecessor
                    # overlapped the second evac, instead of one 1092ns DMA
                    # strictly after the full 1465ns evac
                    ya1 = ya_pool.tile([128, 2, N], ydt, tag="yspl_a", bufs=1)
                    nc.scalar.activation(ya1[:], ps_a[:, 0:2, :], AF.Copy,
                                         scale=osb)
                    nc.sync.dma_start(yrow[:, 0:2 * N],
                                      ya1[:].rearrange("p b n -> p (b n)"))
                    ya2 = ya_pool.tile([128, 1, N], ydt, tag="yspl_b", bufs=1)
                    nc.scalar.activation(ya2[:], ps_a[:, 2:3, :], AF.Copy,
                                         scale=osb)
                    nc.sync.dma_start(yrow[:, 2 * N:3 * N],
                                      ya2[:].rearrange("p b n -> p (b n)"))
                else:
                    ya = ya_pool.tile([128, ab, N], ydt, tag="ysta")
                    nc.scalar.activation(ya[:], ps_a[:], AF.Copy, scale=osb)
                    nc.sync.dma_start(yrow[:, 0:ab * N],
                                      ya[:].rearrange("p b n -> p (b n)"))
    nc.compile()
    return nc


def _get_nc():
    global _cached_nc
    if _cached_nc is None:
        _cached_nc = build_bass()
    return _cached_nc


def _make_in_maps(x: np.ndarray, W: np.ndarray):
    wt = np.ascontiguousarray(W.T)                # [K, N]
    # xt_blk[i, p, c*MT+m] = x[core*M_SH + i*MT + m, c*128 + p]
    xs = x.reshape(N_CORES, N_CHUNKS, MT, KC, 128)
    in_maps = []
    for c in range(N_CORES):
        blk = np.ascontiguousarray(
            xs[c].transpose(0, 3, 2, 1).reshape(N_CHUNKS, 128, KC * MT))
        in_maps.append({"xt": blk, "wt": wt})
    return in_maps


def kernel(x: np.ndarray, W: np.ndarray) -> np.ndarray:
    x = np.ascontiguousarray(x, dtype=np.float32)
    W = np.ascontiguousarray(W, dtype=np.float32)
    assert x.shape == (M_FULL, K) and W.shape == (N, K)

    in_maps = _make_in_maps(x, W)
    nc = _get_nc()
    res = run_bass_kernel_spmd(nc, in_maps, core_ids=list(range(N_CORES)))
    # y_blk[g, p, b*N+n] = y[g*512 + b*128 + p, n]
    outs = []
    for r in res.results:
        yb = r["y"].astype(np.float32).reshape(N_CHUNKS, 128, 4, N)
        outs.append(yb.transpose(0, 2, 1, 3).reshape(M_SH, N))
    return np.ascontiguousarray(np.concatenate(outs, axis=0),
                                dtype=np.float32)
